# revision 14
# baseline (speedup 1.0000x reference)
"""GAT message-passing layer (segment softmax + weighted scatter) on 8 trn2 cores.

Strategy (per spec sharding_hint: 1D-partition destination nodes):
  - Destination nodes are split evenly across the 8 cores (1250 each); every
    edge is routed to the core that owns its destination, so each core runs
    fully independently (no collectives).
  - Host-side (integer index work only): each core's edges are bucketed by
    (destination window of W=28 consecutive destinations) x (source quarter,
    since dma_gather indices are int16) and packed densely into 128-edge
    tiles; a per-edge one-hot over the window is precomputed.  Tiles are
    ordered source-quarter-major so every K-tile chunk gathers from a single
    sub-table with one dma_gather call.
  - Device-side per tile: gather the 128 source rows (bf16 copy of h_sent),
    compute per-edge attention logits with DVE mul+reduce against the
    replicated w_src vector, add the per-edge w_dst score (built from the
    one-hot), leaky-relu + exp, scale rows by the edge weight, and one-hot
    matmul-accumulate into a PSUM tile holding [28 destinations x
    (128 features + denominator)].  Softmax normalization happens once per
    destination at the end (numerator / denominator), so no per-edge alpha is
    ever materialized.
"""

import math
import os
import sys

import numpy as np

for _p in ("/opt/trn_rl_repo", "/root/.axon_site/_ro/trn_rl_repo"):
    if os.path.isdir(_p) and _p not in sys.path:
        sys.path.insert(0, _p)

import ml_dtypes  # noqa: E402

import concourse.bacc as bacc  # noqa: E402
import concourse.bass as bass  # noqa: E402
import concourse.mybir as mybir  # noqa: E402
import concourse.tile as tile  # noqa: E402

F32 = mybir.dt.float32
BF16 = mybir.dt.bfloat16
I16 = mybir.dt.int16
U8 = mybir.dt.uint8
BF = ml_dtypes.bfloat16

N_SENT = 100000
N_TYPE = 10000
D = 128
N_CORES = 8
LEAKY = 0.01

P = 128          # SBUF partitions (edges per tile)
W = 28           # destinations per window (PSUM partition dim)
S = 4            # source sub-tables (dma_gather idx is int16)
K = 16           # tiles per chunk
G = 1024         # rows per dma_gather call (SWDGE ring capacity limit)
NQ = 2           # SWDGE queues to spread gather calls over


def _plan(src_idx, dst_idx, n_sent=N_SENT, n_type=N_TYPE, n_cores=N_CORES):
    """Bucket edges into the (window x source-quarter) tile layout.

    Pure integer index manipulation. Returns per-core device input arrays and
    the static schedule (shared by all cores so one program serves all 8).
    """
    dpc = n_type // n_cores           # destinations per core
    nw = (dpc + W - 1) // W           # windows per core
    sub = (n_sent + S - 1) // S       # rows per source sub-table
    assert sub <= 32767
    deg = np.bincount(dst_idx, minlength=n_type)
    stab = (src_idx // sub).astype(np.int64)          # sub-table of each edge
    dsti = dst_idx.astype(np.int64)
    gkey = ((dsti // dpc) * nw + (dsti % dpc) // W) * S + stab  # (core, window, sub)
    order = np.lexsort((src_idx, gkey))
    src_sorted = src_idx[order].astype(np.int64)
    dst_sorted = dst_idx[order].astype(np.int64)
    gcnt = np.bincount(gkey, minlength=n_cores * nw * S).astype(np.int64)
    gstart = np.zeros(len(gcnt) + 1, np.int64)
    gstart[1:] = np.cumsum(gcnt)

    # tiles per (w, s): max over cores
    M = np.zeros((nw, S), np.int64)
    for w in range(nw):
        for s in range(S):
            for c in range(n_cores):
                gi = (c * nw + w) * S + s
                M[w, s] = max(M[w, s], -(-gcnt[gi] // P))

    # tile order: subtable-major, window-minor; each subtable group padded to K
    tile_base = np.zeros((S, nw), np.int64)   # first tile of segment (s, w)
    segs = []                                  # (s, w, t0, m) with m > 0
    t = 0
    grp_tiles = np.zeros(S, np.int64)
    for s in range(S):
        for w in range(nw):
            tile_base[s, w] = t
            m = int(M[w, s])
            if m:
                segs.append((s, w, t, m))
                t += m
        pad = (-t) % K
        t += pad
        grp_tiles[s] = t
    n_tiles_pad = t
    n_chunks = n_tiles_pad // K
    sub_of_chunk = np.searchsorted(grp_tiles, np.arange(n_chunks) * K, side="right")

    cores = []
    for c in range(n_cores):
        idx16 = np.zeros((16, n_chunks * K * 8), np.int16)
        onehot = np.zeros((P, n_tiles_pad, W), np.float32)
        maskw = np.zeros((W, nw), np.float32)
        for w in range(nw):
            lim = min(W, dpc - w * W)
            for s in range(S):
                gi = (c * nw + w) * S + s
                g0, g1 = gstart[gi], gstart[gi + 1]
                srcs = src_sorted[g0:g1] - s * sub
                dsts = dst_sorted[g0:g1] - (c * dpc + w * W)
                t0 = int(tile_base[s, w])
                pos = np.arange(len(srcs))
                slot = t0 * P + pos          # flat slot id = tile*128 + p
                onehot[slot % P, slot // P, dsts] = 1.0
                idx16[slot % 16, slot // 16] = srcs
                present = np.zeros(W, bool)
                present[np.unique(dsts)] = True
                maskw[:lim, w] = np.maximum(maskw[:lim, w], present[:lim])
        cores.append({
            "idx16": np.ascontiguousarray(np.tile(idx16, (8, 1))),
            "onehot": onehot.reshape(P, n_tiles_pad * W).astype(BF),
            "maskw": maskw,
            "imaskw": (1.0 - maskw).astype(np.float32),
        })

    return {
        "dpc": dpc, "nw": nw, "sub": sub, "segs": segs, "M": M,
        "tile_base": tile_base, "n_tiles_pad": n_tiles_pad,
        "n_chunks": n_chunks, "sub_of_chunk": sub_of_chunk, "cores": cores,
    }


def _offsets(plan):
    """Byte offsets inside the packed per-partition consts (uint8 [128, *])."""
    nch = plan["n_chunks"]
    o = {}
    o["w1"] = 0                               # [128, D] bf16
    o["idx"] = 2 * D                          # [128, nch*K*8] i16
    o["endP"] = o["idx"] + 2 * nch * K * 8
    nw = plan["nw"]
    q = {}                                    # f32 [W, *] -- bf16 for ht/w2
    q["ht"] = 0                               # [W, nw*D] bf16 (zero-padded)
    q["w2"] = nw * D                          # [W, D] bf16
    q["mask"] = q["w2"] + D                   # [W, nw] bf16 (exact 0/1)
    q["imask"] = q["mask"] + nw               # [W, nw] bf16
    q["endQ"] = q["imask"] + nw
    return o, q


def _build(plan, n_sent=N_SENT):
    dpc, nw, sub = plan["dpc"], plan["nw"], plan["sub"]
    n_tiles_pad, n_chunks = plan["n_tiles_pad"], plan["n_chunks"]
    segs, sub_of_chunk = plan["segs"], plan["sub_of_chunk"]
    nw_full = dpc // W
    rem = dpc - nw_full * W
    CW = D + 1                 # psum columns: 128 features + denominator
    RHS = D + 2                # rhs row stride
    WREP = nw * W
    OFF, OQ = _offsets(plan)

    # per-tile: (segment windows w, is_first, is_last) or None for pad tiles
    tinfo = [None] * n_tiles_pad
    for (s, w, t0, m) in segs:
        for i in range(m):
            tinfo[t0 + i] = (w, i == 0, i == m - 1)

    nc = bacc.Bacc(None, target_bir_lowering=False, debug=False,
                   num_swdge_queues=NQ)
    h16_d = nc.dram_tensor("h16", [n_sent, D], BF16, kind="ExternalInput")
    constsP_d = nc.dram_tensor("constsP", [P, OFF["endP"]], U8, kind="ExternalInput")
    constsQ_d = nc.dram_tensor("constsQ", [W, OQ["endQ"]], BF16, kind="ExternalInput")
    oh_d = nc.dram_tensor("onehot", [P, n_tiles_pad * W], BF16, kind="ExternalInput")
    out_d = nc.dram_tensor("out_local", [dpc, D], F32, kind="ExternalOutput")
    sd_scr = nc.dram_tensor("sd_scratch", [1, WREP], F32)

    with tile.TileContext(nc) as tc:
        with (
            tc.tile_pool(name="const", bufs=1) as const,
            tc.tile_pool(name="work", bufs=3) as work,
            tc.tile_pool(name="scratch", bufs=1) as scratch,
            tc.tile_pool(name="psum", bufs=4, space="PSUM") as psum,
        ):
            # ---- one DMA per packed const tensor ----
            cp = const.tile([P, OFF["endP"]], U8)
            nc.sync.dma_start(out=cp[:], in_=constsP_d[:, :])
            w1t = cp[:, OFF["w1"]:OFF["idx"]].bitcast(BF16)          # [128, D]
            idx16 = cp[:, OFF["idx"]:OFF["endP"]].bitcast(I16)       # [128, nch*K*8]

            cq = const.tile([W, OQ["endQ"]], BF16)
            nc.sync.dma_start(out=cq[:], in_=constsQ_d[:, :])
            ht3 = cq[:, OQ["ht"]:OQ["w2"]].rearrange("p (w f) -> p w f", f=D)
            w2t = cq[:, OQ["w2"]:OQ["mask"]]                         # [W, D]
            maskt = cq[:, OQ["mask"]:OQ["imask"]]                    # [W, nw]
            imaskt = cq[:, OQ["imask"]:OQ["endQ"]]                   # [W, nw]

            # ---- s_dst[d] = h_type[d] . w2, in window layout ----
            sdtmp = scratch.tile([W, nw * D], BF16, tag="wideb")
            sdtmp3 = sdtmp[:].rearrange("p (w f) -> p w f", f=D)
            w2b = w2t.rearrange("p (a f) -> p a f", a=1).to_broadcast([W, nw, D])
            nc.vector.tensor_tensor(out=sdtmp3, in0=ht3, in1=w2b, op=mybir.AluOpType.mult)
            sd = scratch.tile([W, nw], F32)
            nc.vector.tensor_reduce(out=sd[:], in_=sdtmp3, axis=mybir.AxisListType.X,
                                    op=mybir.AluOpType.add)
            # roundtrip through DRAM to reorder [W, nw] -> flat row, then
            # replicate across the 128 partitions with a K=1 matmul
            nc.sync.dma_start(
                out=sd_scr[0, 0:WREP].rearrange("(w r) -> r w", r=W), in_=sd[:, :],
            )
            sdrow = scratch.tile([1, WREP], F32)
            nc.sync.dma_start(out=sdrow[:], in_=sd_scr[0:1, :])
            ones = const.tile([1, P], F32)
            nc.vector.memset(ones[:], 1.0)
            sdrep = const.tile([P, WREP], F32)
            for i in range(math.ceil(WREP / 512)):
                n = min(512, WREP - i * 512)
                pt = psum.tile([P, 512], F32, tag="rep")
                nc.tensor.matmul(out=pt[:, 0:n], lhsT=ones[:],
                                 rhs=sdrow[:, i * 512:i * 512 + n], start=True, stop=True)
                nc.vector.tensor_copy(out=sdrep[:, i * 512:i * 512 + n], in_=pt[:, 0:n])
            sdrep3 = sdrep[:].rearrange("p (w r) -> p w r", r=W)

            # ---- window accumulator (features + denominator per dst) ----
            acc = const.tile([W, nw * CW], F32)
            acc3 = acc[:].rearrange("p (w c) -> p w c", c=CW)
            nc.vector.memset(acc[:], 0.0)

            w1b = w1t.rearrange("p (a f) -> p a f", a=1).to_broadcast([P, K, D])
            cur_psum = None

            # chunk -> list of (col0, col1, w) sdexp segments; cols rel to chunk
            chunk_segs = [[] for _ in range(n_chunks)]
            for (s, w, t0, m) in segs:
                for t in range(t0, t0 + m):
                    ch = t // K
                    j = t - ch * K
                    if chunk_segs[ch] and chunk_segs[ch][-1][2] == w \
                            and chunk_segs[ch][-1][1] == j:
                        chunk_segs[ch][-1][1] = j + 1
                    else:
                        chunk_segs[ch].append([j, j + 1, w])

            # ---- main loop over chunks of K tiles ----
            for ch in range(n_chunks):
                t0 = ch * K
                st = int(sub_of_chunk[ch])
                r0 = st * sub
                r1 = min(r0 + sub, n_sent)
                hbuf = work.tile([P, K * D], BF16, tag="hbuf")
                hb3v = hbuf[:].rearrange("p (k f) -> p k f", f=D)
                tpg = G // P                      # tiles per gather call
                for q in range(K * P // G):
                    c0 = ch * K * 8 + q * (G // 16)
                    nc.gpsimd.dma_gather(
                        out_ap=hb3v[:, q * tpg:(q + 1) * tpg, :],
                        in_ap=h16_d[r0:r1, :],
                        idxs_ap=idx16[:, c0:c0 + G // 16],
                        num_idxs=G, num_idxs_reg=G, elem_size=D,
                        queue_num=(ch * (K * P // G) + q) % NQ,
                    )
                oht = work.tile([P, K * W], BF16, tag="oht")
                nc.sync.dma_start(out=oht[:], in_=oh_d[:, t0 * W:(t0 + K) * W])
                oh3 = oht[:].rearrange("p (t r) -> p t r", r=W)

                cols = work.tile([P, 6 * K], F32, tag="cols")
                sdexp = cols[:, 0:K]
                scol = cols[:, K:2 * K]
                tcol = cols[:, 2 * K:3 * K]
                ucol = cols[:, 3 * K:4 * K]
                xcol = cols[:, 4 * K:5 * K]
                xc16 = cols[:, 5 * K:5 * K + K].bitcast(BF16)[:, 0:K]

                # per-edge dst score from the one-hot (zero on pad slots)
                if len(chunk_segs[ch]) == 0 or chunk_segs[ch][0][0] != 0 \
                        or chunk_segs[ch][-1][1] != K:
                    nc.vector.memset(sdexp, 0.0)
                sdxc = work.tile([P, K * W], F32, tag="sdxc")
                sdxc3 = sdxc[:].rearrange("p (t r) -> p t r", r=W)
                for (j0, j1, w) in chunk_segs[ch]:
                    sdwb = (sdrep3[:, w, :].rearrange("p (a r) -> p a r", a=1)
                            .to_broadcast([P, j1 - j0, W]))
                    nc.vector.tensor_tensor(out=sdxc3[:, j0:j1, :], in0=oh3[:, j0:j1, :],
                                            in1=sdwb, op=mybir.AluOpType.mult)
                    nc.vector.tensor_reduce(out=sdexp[:, j0:j1], in_=sdxc3[:, j0:j1, :],
                                            axis=mybir.AxisListType.X,
                                            op=mybir.AluOpType.add)

                # per-edge src score: s = h_src . w1  (mul + row reduce)
                mt = work.tile([P, K * D], BF16, tag="mt")
                mt3 = mt[:].rearrange("p (k f) -> p k f", f=D)
                nc.vector.tensor_tensor(out=mt3, in0=hb3v, in1=w1b, op=mybir.AluOpType.mult)
                nc.vector.tensor_reduce(out=scol, in_=mt3, axis=mybir.AxisListType.X,
                                        op=mybir.AluOpType.add)
                # x = exp(leaky_relu(s + s_dst))
                nc.vector.tensor_tensor(out=tcol, in0=scol, in1=sdexp,
                                        op=mybir.AluOpType.add)
                nc.vector.tensor_scalar_mul(out=ucol, in0=tcol, scalar1=LEAKY)
                nc.vector.tensor_tensor(out=ucol, in0=ucol, in1=tcol,
                                        op=mybir.AluOpType.max)
                nc.scalar.activation(out=xcol, in_=ucol,
                                     func=mybir.ActivationFunctionType.Exp)
                nc.vector.tensor_copy(out=xc16, in_=xcol)

                # rhs rows: [x * h_src, x]
                rhs = work.tile([P, K * RHS], BF16, tag="rhs")
                rhs3 = rhs[:].rearrange("p (k c) -> p k c", c=RHS)
                xb = xc16.to_broadcast([P, K, D])
                nc.vector.tensor_tensor(out=rhs3[:, :, 0:D], in0=hb3v, in1=xb,
                                        op=mybir.AluOpType.mult)
                nc.vector.tensor_copy(out=rhs3[:, :, D], in_=xc16)

                # one-hot matmul accumulate; evacuate-add on segment close
                for j in range(K):
                    ti = tinfo[t0 + j]
                    if ti is None:
                        continue
                    w, first, last = ti
                    if first:
                        cur_psum = psum.tile([W, CW], F32, tag="pw")
                    nc.tensor.matmul(out=cur_psum[:], lhsT=oh3[:, j, :],
                                     rhs=rhs3[:, j, 0:CW], start=first, stop=last)
                    if last:
                        nc.vector.tensor_tensor(out=acc3[:, w, :], in0=acc3[:, w, :],
                                                in1=cur_psum[:], op=mybir.AluOpType.add)

            # ---- finalize: out = mask * num/den + (1-mask) * h_type ----
            den = acc3[:, :, D]
            dadj = scratch.tile([W, nw], F32)
            nc.vector.tensor_tensor(out=dadj[:], in0=den, in1=imaskt, op=mybir.AluOpType.add)
            rec = scratch.tile([W, nw], F32)
            nc.vector.reciprocal(out=rec[:], in_=dadj[:])
            num3 = acc3[:, :, 0:D]
            nc.vector.tensor_tensor(out=num3, in0=num3,
                                    in1=rec[:].to_broadcast([W, nw, D]),
                                    op=mybir.AluOpType.mult)
            nc.vector.tensor_tensor(out=num3, in0=num3,
                                    in1=maskt.to_broadcast([W, nw, D]),
                                    op=mybir.AluOpType.mult)
            httmp = scratch.tile([W, nw * D], BF16, tag="wideb")
            httmp3 = httmp[:].rearrange("p (w f) -> p w f", f=D)
            nc.vector.tensor_tensor(out=httmp3, in0=ht3,
                                    in1=imaskt.to_broadcast([W, nw, D]),
                                    op=mybir.AluOpType.mult)
            nc.vector.tensor_tensor(out=num3, in0=num3, in1=httmp3, op=mybir.AluOpType.add)
            nc.sync.dma_start(
                out=out_d[0:nw_full * W, :].rearrange("(w p) f -> p w f", p=W),
                in_=num3[:, 0:nw_full, :],
            )
            if rem:
                nc.sync.dma_start(out=out_d[nw_full * W:, :], in_=num3[0:rem, nw_full, :])

    nc.finalize()
    return nc


def _in_maps(plan, h_sent, h_type, attn_w):
    dpc, nw = plan["dpc"], plan["nw"]
    OFF, OQ = _offsets(plan)
    w1rep = np.ascontiguousarray(
        np.broadcast_to(attn_w[0, :D].astype(BF), (P, D)))
    w2rep = np.ascontiguousarray(
        np.broadcast_to(attn_w[0, D:].astype(BF), (W, D)))
    h16 = np.ascontiguousarray(h_sent.astype(BF))
    maps = []
    for c, arrs in enumerate(plan["cores"]):
        cp = np.zeros((P, OFF["endP"]), np.uint8)
        cp[:, OFF["w1"]:OFF["idx"]] = w1rep.view(np.uint8)
        cp[:, OFF["idx"]:OFF["endP"]] = arrs["idx16"].view(np.uint8)
        cq = np.zeros((W, OQ["endQ"]), BF)
        htp = np.zeros((nw * W, D), np.float32)
        htp[0:dpc] = h_type[c * dpc:(c + 1) * dpc]
        cq[:, OQ["ht"]:OQ["w2"]] = (htp.reshape(nw, W, D).transpose(1, 0, 2)
                                    .reshape(W, nw * D)).astype(BF)
        cq[:, OQ["w2"]:OQ["mask"]] = w2rep
        cq[:, OQ["mask"]:OQ["imask"]] = arrs["maskw"].astype(BF)
        cq[:, OQ["imask"]:OQ["endQ"]] = arrs["imaskw"].astype(BF)
        maps.append({"h16": h16, "constsP": cp, "constsQ": cq,
                     "onehot": arrs["onehot"]})
    return maps


def prepare(h_sent, h_type, attn_w, src_idx, dst_idx):
    plan = _plan(np.asarray(src_idx), np.asarray(dst_idx))
    nc = _build(plan)
    maps = _in_maps(plan, np.asarray(h_sent), np.asarray(h_type), np.asarray(attn_w))
    return plan, nc, maps


def kernel(h_sent, h_type, attn_w, src_idx, dst_idx):
    from concourse.bass_utils import run_bass_kernel_spmd

    plan, nc, maps = prepare(h_sent, h_type, attn_w, src_idx, dst_idx)
    res = run_bass_kernel_spmd(nc, maps, list(range(N_CORES)))
    out = np.concatenate([r["out_local"] for r in res.results], axis=0)
    return out.astype(np.float32)
